# revision 44
# baseline (speedup 1.0000x reference)
"""Trainium2 Bass kernel for a 2-layer GCN (nn_EvenLamerGCN) - block-major.

reference semantics (PyG GCNConv x2, eval mode):
    deg[i]  = 1 + indeg(i)                (self-loops added)
    dinv    = deg ** -0.5
    h  = relu(A_hat @ (x @ W1) + b1),  A_hat = D^-1/2 (A + I) D^-1/2
    o  = A_hat @ (h @ W2) + b2
    return o, log_softmax(o, axis=1)

Distribution: nodes sharded over 8 NeuronCores (12500/core, padded to
12544), edges partitioned by destination core.  The per-edge norm is
folded into per-node row scalings:
    out = dinv * ( sum_{e: dst=i} T[src_e] + T[i] ),   T = dinv * (x @ W)

Per layer on each core, processed PER 128-DST BLOCK (block-major):
  1. dense matmul -> row-scaled table shard T_c, AllGather -> full T
  2. per block: one dma_gather per (block, src-window) cell with the
     cell's true edge count in num_idxs_reg (descriptor count == real
     edges; trailing -1 padding is trimmed consistently on both the
     sequencer and Q7 sides), round-robin over the 4 SWDGE queues so
     each DMA engine has several outstanding 256B reads
  3. one PSUM group per block: identity matmul adds the self-loop row,
     then one one-hot matmul per gathered chunk
  4. the per-block epilogue (layer 1: relu/scale + W2 matmul into the
     layer-2 table; layer 2: bias + log_softmax + output stores) runs
     in the shadow of the next blocks' gathers.
Instruction streams are identical on all 8 cores (SPMD, one NEFF); all
per-core variation lives in input data (including the per-cell counts
read into num_idxs_reg).
"""

import sys

for _p in ("/opt/trn_rl_repo", "/root/.axon_site/_ro/trn_rl_repo"):
    if _p not in sys.path:
        sys.path.insert(0, _p)

from contextlib import ExitStack
from dataclasses import dataclass

import numpy as np

import concourse.bass as bass
import concourse.mybir as mybir
import concourse.tile as tile
from concourse import bacc
from concourse.bass import ds, ts
from concourse.bass_utils import run_bass_kernel_spmd
from concourse.masks import make_identity

F32 = mybir.dt.float32
BF16 = mybir.dt.bfloat16
I16 = mybir.dt.int16
I32 = mybir.dt.int32
AF = mybir.ActivationFunctionType
ALU = mybir.AluOpType


@dataclass(frozen=True)
class Cfg:
    n: int = 100000          # nodes
    din: int = 512           # input features
    dh: int = 128            # hidden features
    dout: int = 40           # output features
    cores: int = 8
    wsize: int = 32768       # int16 gather window (rows)
    max_piece: int = 32      # iota free-dim capacity (chunks)

    @property
    def nsh(self):           # real nodes per core
        return self.n // self.cores

    @property
    def nloc(self):          # padded nodes per core (multiple of 128)
        return ((self.nsh + 127) // 128) * 128

    @property
    def nt(self):            # 128-node dst blocks per core
        return self.nloc // 128

    @property
    def trows(self):         # rows in the gathered tables
        return self.cores * self.nloc

    @property
    def dh2(self):           # layer-2 compute/output width
        return self.dout

    @property
    def dt2(self):           # layer-2 bf16 table row width (256B rows)
        return max(128, self.dh2)

    @property
    def kt(self):            # k-tiles in the first matmul
        return self.din // 128

    @property
    def nwin(self):          # number of static src windows
        return max(1, -(-self.trows // self.wsize))

    @property
    def wbases(self):
        return [min(w * self.wsize, self.trows - self.wsize)
                for w in range(self.nwin)]


@dataclass(frozen=True)
class Plan:
    quotas: tuple          # chunks per (window) cell, per dst block

    @property
    def cpb(self):         # gathered chunks per block
        return sum(self.quotas)

    @property
    def total_chunks(self):
        return self.cpb * 98  # overwritten by property users via cfg; see use


# ----------------------------------------------------------------------------
# CPU-side preprocessing
# ----------------------------------------------------------------------------

def preprocess(cfg: Cfg, edge_index: np.ndarray):
    c = cfg
    src = np.asarray(edge_index[0], dtype=np.int64)
    dst = np.asarray(edge_index[1], dtype=np.int64)

    deg = np.bincount(dst, minlength=c.n).astype(np.float32) + 1.0
    deg_pt = np.ones((c.cores, 128, c.nt), np.float32)
    for ci in range(c.cores):
        dl = np.ones(c.nloc, np.float32)
        dl[: c.nsh] = deg[ci * c.nsh : (ci + 1) * c.nsh]
        deg_pt[ci] = dl.reshape(c.nt, 128).T

    row_of = lambda i: (i // c.nsh) * c.nloc + (i % c.nsh)
    r_all = row_of(src)
    w_all = np.minimum(r_all // c.wsize, c.nwin - 1)
    core_all = dst // c.nsh
    dloc_all = dst - core_all * c.nsh
    b_all = dloc_all // 128
    id_all = dloc_all % 128

    cell_key = (core_all * c.nt + b_all) * c.nwin + w_all
    counts = np.bincount(cell_key, minlength=c.cores * c.nt * c.nwin)
    counts = counts.reshape(c.cores, c.nt, c.nwin)
    quotas = tuple(int(-(-counts[:, :, w].max() // 128)) for w in range(c.nwin))
    plan = Plan(quotas=quotas)

    bases = c.wbases
    cpb128 = sum(quotas) * 128
    offw = np.concatenate([[0], np.cumsum(quotas)]) * 128
    slots = c.nt * cpb128
    total_chunks = slots // 128

    idx16 = np.zeros((c.cores, 128, slots // 16), np.int16)
    ids_f32 = np.empty((c.cores, 128, total_chunks), np.float32)
    cnts_wm = np.zeros((c.cores, c.nwin * c.nt), np.int32)

    order = np.lexsort((r_all, w_all, b_all, core_all))
    so_r, so_w, so_b, so_core, so_id = (
        r_all[order], w_all[order], b_all[order], core_all[order], id_all[order]
    )
    core_starts = np.searchsorted(so_core, np.arange(c.cores + 1))

    for ci in range(c.cores):
        lo, hi = core_starts[ci], core_starts[ci + 1]
        rr, ii = so_r[lo:hi], so_id[lo:hi]
        cnts_wm[ci] = counts[ci].T.reshape(-1)
        rel = np.zeros(slots, np.int64)      # window-relative gather rows
        ids = np.full(slots, -1.0, np.float32)
        pos = 0
        # block-major: cells of block b at b*cpb128 + offw[w].
        # quota>1 cells: trailing padding -1 (trimmed via num_idxs_reg);
        # quota==1 cells keep padding 0 (full static gather, safe row)
        for b in range(c.nt):
            for w in range(c.nwin):
                cnt = counts[ci, b, w]
                off = b * cpb128 + offw[w]
                if quotas[w] > 1:
                    rel[off + cnt : off + quotas[w] * 128] = -1
                if cnt:
                    rel[off : off + cnt] = rr[pos : pos + cnt] - bases[w]
                    ids[off : off + cnt] = ii[pos : pos + cnt]
                    pos += cnt
        assert pos == hi - lo
        assert rel.max() < c.wsize

        v = rel.reshape(-1, 16)              # slot i at [i%16, i//16]
        wrapped = np.ascontiguousarray(v.T)  # [16, slots/16]
        idx16[ci] = np.tile(wrapped, (8, 1)).astype(np.int16)
        ids_f32[ci] = ids.reshape(total_chunks, 128).T

    return deg_pt, idx16, ids_f32, cnts_wm, plan


# ----------------------------------------------------------------------------
# Device kernel
# ----------------------------------------------------------------------------

def build(nc, tc, cfg: Cfg, plan: Plan):
    c = cfg
    RG = [list(range(c.cores))]
    quotas = plan.quotas
    cpb = sum(quotas)
    offw = [0]
    for qv in quotas:
        offw.append(offw[-1] + qv)
    slots = c.nt * cpb * 128
    total_chunks = slots // 128

    x_sh = nc.dram_tensor("x_sh", [c.din, c.nloc], BF16, kind="ExternalInput").ap()
    w1 = nc.dram_tensor("w1", [c.din, c.dh], BF16, kind="ExternalInput").ap()
    w2 = nc.dram_tensor("w2", [c.dh, c.dh2], BF16, kind="ExternalInput").ap()
    b1r = nc.dram_tensor("b1r", [128, c.dh], F32, kind="ExternalInput").ap()
    b2r = nc.dram_tensor("b2r", [128, c.dh2], F32, kind="ExternalInput").ap()
    degp = nc.dram_tensor("degp", [128, c.nt], F32, kind="ExternalInput").ap()
    idx16 = nc.dram_tensor("idx16", [128, slots // 16], I16, kind="ExternalInput").ap()
    idsf = nc.dram_tensor("idsf", [128, total_chunks], BF16, kind="ExternalInput").ap()
    cnts = nc.dram_tensor("cnts32", [128, c.nwin * c.nt], I32, kind="ExternalInput").ap()
    out_h = nc.dram_tensor("out_h", [c.nloc, c.dh2], F32, kind="ExternalOutput").ap()
    out_ls = nc.dram_tensor("out_ls", [c.nloc, c.dh2], F32, kind="ExternalOutput").ap()

    t1_loc = nc.dram_tensor("t1_loc", [c.nloc, c.dh], BF16, kind="Internal").ap()
    t1_full = nc.dram_tensor(
        "t1_full", [c.trows, c.dh], BF16, kind="Internal", addr_space="Shared"
    ).ap()
    t2_loc = nc.dram_tensor("t2_loc", [c.nloc, c.dt2], BF16, kind="Internal").ap()
    t2_full = nc.dram_tensor(
        "t2_full", [c.trows, c.dt2], BF16, kind="Internal", addr_space="Shared"
    ).ap()

    with ExitStack() as st:
        cpool = st.enter_context(tc.tile_pool(name="consts", bufs=1))
        gp = st.enter_context(tc.tile_pool(name="gp", bufs=6))
        sp = st.enter_context(tc.tile_pool(name="sp", bufs=3))
        pp = st.enter_context(tc.tile_pool(name="pp", bufs=4))
        ppsum = st.enter_context(tc.tile_pool(name="ppsum", bufs=3, space="PSUM"))
        p0 = st.enter_context(tc.tile_pool(name="p0", bufs=3))
        p0ps = st.enter_context(tc.tile_pool(name="p0ps", bufs=2, space="PSUM"))
        p0psT = st.enter_context(tc.tile_pool(name="p0psT", bufs=1, space="PSUM"))

        # ---- constants ----
        identb = cpool.tile([128, 128], BF16)
        make_identity(nc, identb)
        w1sb = cpool.tile([128, c.kt, c.dh], BF16)
        nc.sync.dma_start(w1sb, w1.rearrange("(o p) f -> p o f", p=128))
        w2sb = cpool.tile([128, c.dh2], BF16)
        nc.sync.dma_start(w2sb, w2)
        b1sb = cpool.tile([128, c.dh], F32)
        nc.sync.dma_start(b1sb, b1r)
        b2sb = cpool.tile([128, c.dh2], F32)
        nc.sync.dma_start(b2sb, b2r)
        dinv = cpool.tile([128, c.nt], F32)
        nc.sync.dma_start(dinv, degp)
        nc.scalar.activation(dinv, dinv, AF.Sqrt)
        nc.vector.reciprocal(dinv, dinv)
        iota = cpool.tile([128, c.max_piece, 128], BF16)
        nc.gpsimd.iota(iota, pattern=[[0, c.max_piece], [1, 128]], base=0,
                       channel_multiplier=0,
                       allow_small_or_imprecise_dtypes=True)
        cntsb = cpool.tile([128, c.nwin * c.nt], I32)
        nc.sync.dma_start(cntsb, cnts)
        creg = nc.alloc_register(mybir.EngineType.Pool, "gcnt")
        self_gq = [0]

        # zero the gather buffers once: reg-trimmed gathers leave padding
        # slots unwritten, and stale NaN-pattern garbage would poison the
        # 0*x one-hot matmul; afterwards stale data is old finite rows
        for _ in range(6):
            gz = gp.tile([128, cpb, 128], BF16, tag="gt")
            nc.vector.memset(gz, 0.0)

        # ---- phase 0: T1 = dinv * (x @ W1), write local table shard ----
        # load 4 node-tiles per DMA (1KB contiguous runs per partition row)
        xv = x_sh.rearrange("(j p) n -> p j n", p=128)
        TG = 4
        for t0 in range(0, c.nt, TG):
            ng = min(TG, c.nt - t0)
            xt = p0.tile([128, c.kt, TG * 128], BF16, tag="xt")
            nc.sync.dma_start(
                xt[:, :, : ng * 128], xv[:, :, ds(t0 * 128, ng * 128)]
            )
            for ti in range(ng):
                t = t0 + ti
                hps = p0ps.tile([128, c.dh], F32, tag="hps")
                for j in range(c.kt):
                    nc.tensor.matmul(
                        hps, lhsT=xt[:, j, ts(ti, 128)], rhs=w1sb[:, j, :],
                        start=(j == 0), stop=(j == c.kt - 1),
                    )
                hsb = p0.tile([128, c.dh], BF16, tag="hsb")
                nc.vector.tensor_scalar_mul(hsb, hps, dinv[:, t : t + 1])
                nc.sync.dma_start(t1_loc[ts(t, 128), :], hsb)

        nc.gpsimd.collective_compute(
            "AllGather", ALU.bypass, replica_groups=RG,
            ins=[t1_loc.opt()], outs=[t1_full.opt()],
        )

        # ---- block-major edge aggregation ----
        OB = 7  # blocks per index-octet load (98 = 14*7)

        def edge_phase(table_full, t_loc, d, dt, post_block):
            tv = t_loc.rearrange("(b p) f -> p b f", p=128)
            for ob in range(0, c.nt, OB):
                nb = min(OB, c.nt - ob)
                sit = sp.tile([128, OB * cpb * 8], I16, tag="sit")
                nc.sync.dma_start(
                    sit[:, : nb * cpb * 8],
                    idx16[:, ob * cpb * 8 : (ob + nb) * cpb * 8],
                )
                sid = sp.tile([128, OB * cpb], BF16, tag="sid")
                nc.sync.dma_start(sid[:, : nb * cpb], idsf[:, ob * cpb : (ob + nb) * cpb])
                for bi in range(nb):
                    b = ob + bi
                    sd = p0.tile([128, dt], BF16, tag="sd")
                    nc.sync.dma_start(sd, tv[:, b, :])
                    g = gp.tile([128, cpb, dt], BF16, tag="gt")
                    for w, qv in enumerate(quotas):
                        if qv > 1:
                            nc.gpsimd.reg_load(
                                creg,
                                cntsb[0:1, w * c.nt + b : w * c.nt + b + 1],
                            )
                            nreg = creg
                        else:
                            nreg = qv * 128
                        nc.gpsimd.dma_gather(
                            g[:, offw[w] : offw[w + 1], :],
                            table_full[ds(c.wbases[w], c.wsize), :],
                            sit[:, (bi * cpb + offw[w]) * 8 : (bi * cpb + offw[w + 1]) * 8],
                            num_idxs=qv * 128, num_idxs_reg=nreg, elem_size=dt,
                            single_packet=False, queue_num=self_gq[0],
                        )
                        self_gq[0] = (self_gq[0] + 1) % 4
                    stt = pp.tile([128, cpb, 128], BF16, tag="stt")
                    nc.vector.tensor_tensor(
                        stt, iota[:, :cpb, :],
                        sid[:, bi * cpb : (bi + 1) * cpb, None].to_broadcast(
                            (128, cpb, 128)
                        ),
                        ALU.is_equal,
                    )
                    ps = ppsum.tile([128, d], F32, tag="ps")
                    nc.tensor.matmul(ps, lhsT=identb, rhs=sd[:, :d],
                                     start=True, stop=False)
                    for j in range(cpb):
                        nc.tensor.matmul(
                            ps, lhsT=stt[:, j, :], rhs=g[:, j, :d],
                            start=False, stop=(j == cpb - 1),
                        )
                    post_block(b, ps)

        # ---- layer 1: aggregate, then per block build the layer-2 table ----
        def post1(b, ps):
            g1 = p0.tile([128, c.dh], F32, tag="g1")
            nc.vector.tensor_scalar_mul(g1, ps, dinv[:, b : b + 1])
            nc.vector.tensor_tensor(g1, g1, b1sb, ALU.add)
            nc.scalar.activation(g1, g1, AF.Relu)
            g1b = p0.tile([128, c.dh], BF16, tag="g1b")
            nc.vector.tensor_scalar_mul(g1b, g1, dinv[:, b : b + 1])
            tps = p0psT.tile([128, 128], BF16, tag="tps")
            nc.tensor.transpose(tps, g1b, identb)
            gT = p0.tile([128, 128], BF16, tag="gT")
            nc.vector.tensor_copy(gT, tps)
            h2ps = p0ps.tile([128, c.dh2], F32, tag="h2ps")
            nc.tensor.matmul(h2ps, lhsT=gT, rhs=w2sb, start=True, stop=True)
            h2sb = p0.tile([128, c.dh2], BF16, tag="h2sb")
            nc.vector.tensor_copy(h2sb, h2ps)
            nc.sync.dma_start(t2_loc[ts(b, 128), : c.dh2], h2sb)

        edge_phase(t1_full, t1_loc, c.dh, c.dh, post1)

        nc.gpsimd.collective_compute(
            "AllGather", ALU.bypass, replica_groups=RG,
            ins=[t2_loc.opt()], outs=[t2_full.opt()],
        )

        # ---- layer 2: aggregate, then per block bias + log_softmax ----
        ohv = out_h.rearrange("(t p) f -> p t f", p=128)
        olv = out_ls.rearrange("(t p) f -> p t f", p=128)

        def post2(b, ps):
            oh = p0.tile([128, c.dh2], F32, tag="oh")
            nc.vector.tensor_scalar_mul(oh, ps, dinv[:, b : b + 1])
            nc.vector.tensor_tensor(oh, oh, b2sb, ALU.add)
            nc.sync.dma_start(ohv[:, b, :], oh)
            mx = p0.tile([128, 1], F32, tag="mx")
            nc.vector.tensor_reduce(mx, oh, mybir.AxisListType.X, ALU.max)
            sm = p0.tile([128, c.dh2], F32, tag="sm")
            nc.vector.tensor_scalar_sub(sm, oh, mx)
            e1 = p0.tile([128, c.dh2], F32, tag="e1")
            nc.scalar.activation(e1, sm, AF.Exp)
            se = p0.tile([128, 1], F32, tag="se")
            nc.vector.tensor_reduce(se, e1, mybir.AxisListType.X, ALU.add)
            ln = p0.tile([128, 1], F32, tag="ln")
            nc.scalar.activation(ln, se, AF.Ln)
            nc.vector.tensor_scalar_sub(sm, sm, ln)
            nc.sync.dma_start(olv[:, b, :], sm)

        edge_phase(t2_full, t2_loc, c.dh2, c.dt2, post2)


# ----------------------------------------------------------------------------
# Host entry point
# ----------------------------------------------------------------------------

_CACHE = {}


def _get_compiled(cfg: Cfg, plan: Plan):
    key = (cfg, plan)
    if key not in _CACHE:
        nc = bacc.Bacc(
            "TRN2", target_bir_lowering=False, debug=False,
            num_devices=cfg.cores, num_swdge_queues=4,
        )
        with tile.TileContext(nc) as tc:
            build(nc, tc, cfg, plan)
        nc.compile()
        _CACHE[key] = nc
    return _CACHE[key]


def make_in_maps(cfg: Cfg, x, W1, b1, W2, b2, deg_pt, idx16, ids_f32, cnts_wm):
    import ml_dtypes

    c = cfg
    x = np.asarray(x, np.float32)
    w2p = np.asarray(W2, np.float32)[:, : c.dh2].astype(ml_dtypes.bfloat16)
    b1rep = np.tile(np.asarray(b1, np.float32)[None, :], (128, 1))
    b2rep = np.tile(np.asarray(b2, np.float32)[None, : c.dh2], (128, 1))
    w1c = np.ascontiguousarray(
        np.asarray(W1, np.float32).astype(ml_dtypes.bfloat16)
    )

    in_maps = []
    for ci in range(c.cores):
        xs = np.zeros((c.din, c.nloc), ml_dtypes.bfloat16)
        xs[:, : c.nsh] = (
            x[ci * c.nsh : (ci + 1) * c.nsh].astype(ml_dtypes.bfloat16).T
        )
        in_maps.append({
            "x_sh": np.ascontiguousarray(xs),
            "w1": w1c,
            "w2": np.ascontiguousarray(w2p),
            "b1r": b1rep,
            "b2r": np.ascontiguousarray(b2rep),
            "degp": np.ascontiguousarray(deg_pt[ci]),
            "idx16": np.ascontiguousarray(idx16[ci]),
            "idsf": np.ascontiguousarray(ids_f32[ci].astype(ml_dtypes.bfloat16)),
            "cnts32": np.ascontiguousarray(
                np.tile(cnts_wm[ci][None, :], (128, 1))
            ),
        })
    return in_maps


def _ensure_ntff_hook():
    """Install the axon NTFF profile hook if the image's antenv lacks it."""
    import types

    try:
        from antenv.axon_hooks import get_axon_ntff_profile_hook  # noqa: F401
        return
    except ImportError:
        pass
    import antenv

    m = types.ModuleType("antenv.axon_hooks")
    m._hook = None
    m.set_axon_ntff_profile_hook = lambda h: setattr(m, "_hook", h)
    m.get_axon_ntff_profile_hook = lambda: m._hook
    sys.modules["antenv.axon_hooks"] = m
    antenv.axon_hooks = m
    try:
        from trn_agent_boot.trn_boot import _ntff_profile_via_ctypes

        h = _ntff_profile_via_ctypes("/opt/axon/libaxon_pjrt.so")
        if h is not None:
            m._hook = h
    except Exception as e:
        print(f"ntff hook install failed: {e}")

    from concourse import bass_utils as bu

    bu.upload_artifacts = lambda tmpdir: tmpdir


def run(cfg: Cfg, inputs: dict, trace: bool = False):
    if trace:
        _ensure_ntff_hook()
    deg_pt, idx16, ids_f32, cnts_wm, plan = preprocess(cfg, inputs["edge_index"])
    nc = _get_compiled(cfg, plan)
    in_maps = make_in_maps(
        cfg, inputs["x"], inputs["W1"], inputs["b1"], inputs["W2"], inputs["b2"],
        deg_pt, idx16, ids_f32, cnts_wm,
    )
    res = run_bass_kernel_spmd(
        nc, in_maps, core_ids=list(range(cfg.cores)), trace=trace
    )
    c = cfg
    h = np.concatenate(
        [res.results[ci]["out_h"][: c.nsh, : c.dout] for ci in range(c.cores)], axis=0
    )
    ls = np.concatenate(
        [res.results[ci]["out_ls"][: c.nsh, : c.dout] for ci in range(c.cores)], axis=0
    )
    return (h, ls), res


def kernel(**inputs):
    (h, ls), _ = run(Cfg(), inputs)
    return h, ls


# revision 52
# speedup vs baseline: 1.0421x; 1.0421x over previous
"""Trainium2 Bass kernel for a 2-layer GCN (nn_EvenLamerGCN) - block-major.

reference semantics (PyG GCNConv x2, eval mode):
    deg[i]  = 1 + indeg(i)                (self-loops added)
    dinv    = deg ** -0.5
    h  = relu(A_hat @ (x @ W1) + b1),  A_hat = D^-1/2 (A + I) D^-1/2
    o  = A_hat @ (h @ W2) + b2
    return o, log_softmax(o, axis=1)

Distribution: nodes sharded over 8 NeuronCores (12500/core, padded to
12544), edges partitioned by destination core.  The per-edge norm is
folded into per-node row scalings:
    out = dinv * ( sum_{e: dst=i} T[src_e] + T[i] ),   T = dinv * (x @ W)

Per layer on each core, processed PER 128-DST BLOCK (block-major):
  1. dense matmul -> row-scaled table shard T_c, AllGather -> full T
  2. per block: one dma_gather per (block, src-window) cell with the
     cell's true edge count in num_idxs_reg (descriptor count == real
     edges; trailing -1 padding is trimmed consistently on both the
     sequencer and Q7 sides), round-robin over the 4 SWDGE queues so
     each DMA engine has several outstanding 256B reads
  3. one PSUM group per block: identity matmul adds the self-loop row,
     then one one-hot matmul per gathered chunk
  4. the per-block epilogue (layer 1: relu/scale + W2 matmul into the
     layer-2 table; layer 2: bias + log_softmax + output stores) runs
     in the shadow of the next blocks' gathers.
Instruction streams are identical on all 8 cores (SPMD, one NEFF); all
per-core variation lives in input data (including the per-cell counts
read into num_idxs_reg).
"""

import sys

for _p in ("/opt/trn_rl_repo", "/root/.axon_site/_ro/trn_rl_repo"):
    if _p not in sys.path:
        sys.path.insert(0, _p)

from contextlib import ExitStack
from dataclasses import dataclass

import numpy as np

import concourse.bass as bass
import concourse.mybir as mybir
import concourse.tile as tile
from concourse import bacc
from concourse.bass import ds, ts
from concourse.bass_utils import run_bass_kernel_spmd
from concourse.masks import make_identity

F32 = mybir.dt.float32
BF16 = mybir.dt.bfloat16
I16 = mybir.dt.int16
I32 = mybir.dt.int32
AF = mybir.ActivationFunctionType
ALU = mybir.AluOpType


@dataclass(frozen=True)
class Cfg:
    n: int = 100000          # nodes
    din: int = 512           # input features
    dh: int = 128            # hidden features
    dout: int = 40           # output features
    cores: int = 8
    wsize: int = 32768       # int16 gather window (rows)
    max_piece: int = 32      # iota free-dim capacity (chunks)

    @property
    def nsh(self):           # real nodes per core
        return self.n // self.cores

    @property
    def nloc(self):          # padded nodes per core (multiple of 128)
        return ((self.nsh + 127) // 128) * 128

    @property
    def nt(self):            # 128-node dst blocks per core
        return self.nloc // 128

    @property
    def trows(self):         # rows in the gathered tables
        return self.cores * self.nloc

    @property
    def dh2(self):           # layer-2 compute/output width
        return self.dout

    @property
    def dt2(self):           # layer-2 bf16 table row width (256B rows)
        return max(128, self.dh2)

    @property
    def kt(self):            # k-tiles in the first matmul
        return self.din // 128

    @property
    def nwin(self):          # number of static src windows
        return max(1, -(-self.trows // self.wsize))

    @property
    def wbases(self):
        return [min(w * self.wsize, self.trows - self.wsize)
                for w in range(self.nwin)]


@dataclass(frozen=True)
class Plan:
    quotas: tuple          # chunks per (window) cell, per dst block

    @property
    def cpb(self):         # gathered chunks per block
        return sum(self.quotas)

    @property
    def total_chunks(self):
        return self.cpb * 98  # overwritten by property users via cfg; see use


# ----------------------------------------------------------------------------
# CPU-side preprocessing
# ----------------------------------------------------------------------------

def preprocess(cfg: Cfg, edge_index: np.ndarray):
    c = cfg
    src = np.asarray(edge_index[0], dtype=np.int64)
    dst = np.asarray(edge_index[1], dtype=np.int64)

    deg = np.bincount(dst, minlength=c.n).astype(np.float32) + 1.0
    deg_pt = np.ones((c.cores, 128, c.nt), np.float32)
    for ci in range(c.cores):
        dl = np.ones(c.nloc, np.float32)
        dl[: c.nsh] = deg[ci * c.nsh : (ci + 1) * c.nsh]
        deg_pt[ci] = dl.reshape(c.nt, 128).T

    # stacked-half table layout: local rows [0, nloc/2) of all cores form
    # global rows [0, trows/2), the upper local halves form the rest.  Each
    # half is a standalone AllGather, so the first half's collective can
    # overlap producing the second half.
    nh2 = c.nloc // 2
    th2 = c.cores * nh2

    def row_of(i):
        cc = i // c.nsh
        l = i % c.nsh
        half = l // nh2
        return half * th2 + cc * nh2 + (l - half * nh2)

    r_all = row_of(src)
    w_all = np.minimum(r_all // c.wsize, c.nwin - 1)
    core_all = dst // c.nsh
    dloc_all = dst - core_all * c.nsh
    b_all = dloc_all // 128
    id_all = dloc_all % 128

    cell_key = (core_all * c.nt + b_all) * c.nwin + w_all
    counts = np.bincount(cell_key, minlength=c.cores * c.nt * c.nwin)
    counts = counts.reshape(c.cores, c.nt, c.nwin)
    quotas = tuple(int(-(-counts[:, :, w].max() // 128)) for w in range(c.nwin))
    plan = Plan(quotas=quotas)

    bases = c.wbases
    cpb128 = sum(quotas) * 128
    offw = np.concatenate([[0], np.cumsum(quotas)]) * 128
    slots = c.nt * cpb128
    total_chunks = slots // 128

    idx16 = np.zeros((c.cores, 128, slots // 16), np.int16)
    ids_f32 = np.empty((c.cores, 128, total_chunks), np.float32)
    cnts_wm = np.zeros((c.cores, c.nwin * c.nt), np.int32)

    order = np.lexsort((r_all, w_all, b_all, core_all))
    so_r, so_w, so_b, so_core, so_id = (
        r_all[order], w_all[order], b_all[order], core_all[order], id_all[order]
    )
    core_starts = np.searchsorted(so_core, np.arange(c.cores + 1))

    for ci in range(c.cores):
        lo, hi = core_starts[ci], core_starts[ci + 1]
        rr, ii = so_r[lo:hi], so_id[lo:hi]
        cnts_wm[ci] = counts[ci].reshape(-1)   # block-major [b][w]
        rel = np.full(slots, -1, np.int64)   # window-relative gather rows
        ids = np.full(slots, -1.0, np.float32)
        pos = 0
        # block-major: cells of block b at b*cpb128 + offw[w].  Every cell
        # is one sub-gather whose true count rides in num_idxs_reg, so all
        # trailing padding is -1 (trimmed consistently on the sequencer and
        # Q7 sides; no descriptors generated for it)
        for b in range(c.nt):
            for w in range(c.nwin):
                cnt = counts[ci, b, w]
                off = b * cpb128 + offw[w]
                if cnt:
                    rel[off : off + cnt] = rr[pos : pos + cnt] - bases[w]
                    ids[off : off + cnt] = ii[pos : pos + cnt]
                    pos += cnt
        assert pos == hi - lo
        assert rel.max() < c.wsize

        v = rel.reshape(-1, 16)              # slot i at [i%16, i//16]
        wrapped = np.ascontiguousarray(v.T)  # [16, slots/16]
        idx16[ci] = np.tile(wrapped, (8, 1)).astype(np.int16)
        ids_f32[ci] = ids.reshape(total_chunks, 128).T

    return deg_pt, idx16, ids_f32, cnts_wm, plan


# ----------------------------------------------------------------------------
# Device kernel
# ----------------------------------------------------------------------------

def build(nc, tc, cfg: Cfg, plan: Plan):
    c = cfg
    RG = [list(range(c.cores))]
    quotas = plan.quotas
    cpb = sum(quotas)
    offw = [0]
    for qv in quotas:
        offw.append(offw[-1] + qv)
    slots = c.nt * cpb * 128
    total_chunks = slots // 128

    x_sh = nc.dram_tensor("x_sh", [c.din, c.nloc], BF16, kind="ExternalInput").ap()
    w1 = nc.dram_tensor("w1", [c.din, c.dh], BF16, kind="ExternalInput").ap()
    w2 = nc.dram_tensor("w2", [c.dh, c.dh2], BF16, kind="ExternalInput").ap()
    b1r = nc.dram_tensor("b1r", [128, c.dh], F32, kind="ExternalInput").ap()
    b2r = nc.dram_tensor("b2r", [128, c.dh2], F32, kind="ExternalInput").ap()
    degp = nc.dram_tensor("degp", [128, c.nt], F32, kind="ExternalInput").ap()
    idx16 = nc.dram_tensor("idx16", [128, slots // 16], I16, kind="ExternalInput").ap()
    idsf = nc.dram_tensor("idsf", [128, total_chunks], BF16, kind="ExternalInput").ap()
    cnts = nc.dram_tensor("cnts32", [128, c.nwin * c.nt], I32, kind="ExternalInput").ap()
    out_h = nc.dram_tensor("out_h", [c.nloc, c.dh2], F32, kind="ExternalOutput").ap()
    out_ls = nc.dram_tensor("out_ls", [c.nloc, c.dh2], F32, kind="ExternalOutput").ap()

    t1_loc = nc.dram_tensor("t1_loc", [c.nloc, c.dh], BF16, kind="Internal").ap()
    t1_full = nc.dram_tensor(
        "t1_full", [c.trows, c.dh], BF16, kind="Internal", addr_space="Shared"
    ).ap()
    t2_loc = nc.dram_tensor("t2_loc", [c.nloc, c.dt2], BF16, kind="Internal").ap()
    t2_full = nc.dram_tensor(
        "t2_full", [c.trows, c.dt2], BF16, kind="Internal", addr_space="Shared"
    ).ap()

    with ExitStack() as st:
        cpool = st.enter_context(tc.tile_pool(name="consts", bufs=1))
        gp = st.enter_context(tc.tile_pool(name="gp", bufs=4))
        sp = st.enter_context(tc.tile_pool(name="sp", bufs=3))
        pp = st.enter_context(tc.tile_pool(name="pp", bufs=3))
        ppsum = st.enter_context(tc.tile_pool(name="ppsum", bufs=3, space="PSUM"))
        p0 = st.enter_context(tc.tile_pool(name="p0", bufs=3))
        p0ps = st.enter_context(tc.tile_pool(name="p0ps", bufs=2, space="PSUM"))
        p0psT = st.enter_context(tc.tile_pool(name="p0psT", bufs=1, space="PSUM"))

        # ---- constants ----
        identb = cpool.tile([128, 128], BF16)
        make_identity(nc, identb)
        w1sb = cpool.tile([128, c.kt, c.dh], BF16)
        nc.sync.dma_start(w1sb, w1.rearrange("(o p) f -> p o f", p=128))
        w2sb = cpool.tile([128, c.dh2], BF16)
        nc.sync.dma_start(w2sb, w2)
        b1sb = cpool.tile([128, c.dh], F32)
        nc.sync.dma_start(b1sb, b1r)
        b2sb = cpool.tile([128, c.dh2], F32)
        nc.sync.dma_start(b2sb, b2r)
        dinv = cpool.tile([128, c.nt], F32)
        nc.sync.dma_start(dinv, degp)
        nc.scalar.activation(dinv, dinv, AF.Sqrt)
        nc.vector.reciprocal(dinv, dinv)
        iota = cpool.tile([128, c.max_piece, 128], BF16)
        nc.gpsimd.iota(iota, pattern=[[0, c.max_piece], [1, 128]], base=0,
                       channel_multiplier=0,
                       allow_small_or_imprecise_dtypes=True)
        cntsb = cpool.tile([128, c.nwin * c.nt], I32)
        nc.sync.dma_start(cntsb, cnts)
        cregs = [
            nc.alloc_register(mybir.EngineType.Pool, f"gcnt{w}")
            for w in range(c.nwin)
        ]
        self_gq = [0]

        # zero the gather buffers once: reg-trimmed gathers leave padding
        # slots unwritten, and stale NaN-pattern garbage would poison the
        # 0*x one-hot matmul; afterwards stale data is old finite rows
        for _ in range(4):
            gz = gp.tile([128, cpb, 128], BF16, tag="gt")
            nc.vector.memset(gz, 0.0)

        # ---- phase 0: T1 = dinv * (x @ W1), write local table shard ----
        xv = x_sh.rearrange("(j p) n -> p j n", p=128)
        for t in range(c.nt):
            xt = p0.tile([128, c.kt, 128], BF16, tag="xt")
            nc.sync.dma_start(xt, xv[:, :, ts(t, 128)])
            hps = p0ps.tile([128, c.dh], F32, tag="hps")
            for j in range(c.kt):
                nc.tensor.matmul(
                    hps, lhsT=xt[:, j, :], rhs=w1sb[:, j, :],
                    start=(j == 0), stop=(j == c.kt - 1),
                )
            hsb = p0.tile([128, c.dh], BF16, tag="hsb")
            nc.vector.tensor_scalar_mul(hsb, hps, dinv[:, t : t + 1])
            nc.sync.dma_start(t1_loc[ts(t, 128), :], hsb)

        # stacked-half AllGathers: the first half's collective runs while the
        # second half of the table is still being produced
        nh2 = c.nloc // 2
        th2 = c.cores * nh2

        def gather_half(t_loc, t_full, half):
            nc.gpsimd.collective_compute(
                "AllGather", ALU.bypass, replica_groups=RG,
                ins=[t_loc[ds(half * nh2, nh2), :].opt()],
                outs=[t_full[ds(half * th2, th2), :].opt()],
            )

        gather_half(t1_loc, t1_full, 0)
        gather_half(t1_loc, t1_full, 1)

        # ---- block-major edge aggregation ----
        OB = 7  # blocks per index-octet load (98 = 14*7)

        def edge_phase(table_full, t_loc, d, dt, post_block, mid_hook=None):
            tv = t_loc.rearrange("(b p) f -> p b f", p=128)
            for ob in range(0, c.nt, OB):
                nb = min(OB, c.nt - ob)
                sit = sp.tile([128, OB * cpb * 8], I16, tag="sit")
                nc.sync.dma_start(
                    sit[:, : nb * cpb * 8],
                    idx16[:, ob * cpb * 8 : (ob + nb) * cpb * 8],
                )
                sid = sp.tile([128, OB * cpb], BF16, tag="sid")
                nc.sync.dma_start(sid[:, : nb * cpb], idsf[:, ob * cpb : (ob + nb) * cpb])
                for bi in range(nb):
                    b = ob + bi
                    sd = p0.tile([128, dt], BF16, tag="sd")
                    nc.sync.dma_start(sd, tv[:, b, :])
                    g = gp.tile([128, cpb, dt], BF16, tag="gt")
                    nc.gpsimd.reg_load(
                        cregs, cntsb[0:1, b * c.nwin : (b + 1) * c.nwin]
                    )
                    for w, qv in enumerate(quotas):
                        nc.gpsimd.dma_gather(
                            g[:, offw[w] : offw[w + 1], :],
                            table_full[ds(c.wbases[w], c.wsize), :],
                            sit[:, (bi * cpb + offw[w]) * 8 : (bi * cpb + offw[w + 1]) * 8],
                            num_idxs=qv * 128, num_idxs_reg=cregs[w],
                            elem_size=dt,
                            single_packet=False, queue_num=self_gq[0],
                        )
                        self_gq[0] = (self_gq[0] + 1) % 4
                    stt = pp.tile([128, cpb, 128], BF16, tag="stt")
                    nc.vector.tensor_tensor(
                        stt, iota[:, :cpb, :],
                        sid[:, bi * cpb : (bi + 1) * cpb, None].to_broadcast(
                            (128, cpb, 128)
                        ),
                        ALU.is_equal,
                    )
                    ps = ppsum.tile([128, d], F32, tag="ps")
                    nc.tensor.matmul(ps, lhsT=identb, rhs=sd[:, :d],
                                     start=True, stop=False)
                    for j in range(cpb):
                        nc.tensor.matmul(
                            ps, lhsT=stt[:, j, :], rhs=g[:, j, :d],
                            start=False, stop=(j == cpb - 1),
                        )
                    post_block(b, ps)
                    if mid_hook is not None and b == 60:
                        mid_hook()

        # ---- layer 1: aggregate, then per block build the layer-2 table ----
        def post1(b, ps):
            g1 = p0.tile([128, c.dh], F32, tag="g1")
            nc.vector.tensor_scalar_mul(g1, ps, dinv[:, b : b + 1])
            nc.vector.tensor_tensor(g1, g1, b1sb, ALU.add)
            nc.scalar.activation(g1, g1, AF.Relu)
            g1b = p0.tile([128, c.dh], BF16, tag="g1b")
            nc.vector.tensor_scalar_mul(g1b, g1, dinv[:, b : b + 1])
            tps = p0psT.tile([128, 128], BF16, tag="tps")
            nc.tensor.transpose(tps, g1b, identb)
            gT = p0.tile([128, 128], BF16, tag="gT")
            nc.vector.tensor_copy(gT, tps)
            h2ps = p0ps.tile([128, c.dh2], F32, tag="h2ps")
            nc.tensor.matmul(h2ps, lhsT=gT, rhs=w2sb, start=True, stop=True)
            h2sb = p0.tile([128, c.dh2], BF16, tag="h2sb")
            nc.vector.tensor_copy(h2sb, h2ps)
            nc.sync.dma_start(t2_loc[ts(b, 128), : c.dh2], h2sb)

        # the first half of the t2 table (blocks 0-48) is complete well
        # before layer 1 finishes: launch its AllGather from inside the
        # layer-1 block loop so it overlaps the remaining blocks
        edge_phase(t1_full, t1_loc, c.dh, c.dh, post1,
                   mid_hook=lambda: gather_half(t2_loc, t2_full, 0))

        gather_half(t2_loc, t2_full, 1)

        # ---- layer 2: aggregate, then per block bias + log_softmax ----
        ohv = out_h.rearrange("(t p) f -> p t f", p=128)
        olv = out_ls.rearrange("(t p) f -> p t f", p=128)

        def post2(b, ps):
            oh = p0.tile([128, c.dh2], F32, tag="oh")
            nc.vector.tensor_scalar_mul(oh, ps, dinv[:, b : b + 1])
            nc.vector.tensor_tensor(oh, oh, b2sb, ALU.add)
            nc.sync.dma_start(ohv[:, b, :], oh)
            mx = p0.tile([128, 1], F32, tag="mx")
            nc.vector.tensor_reduce(mx, oh, mybir.AxisListType.X, ALU.max)
            sm = p0.tile([128, c.dh2], F32, tag="sm")
            nc.vector.tensor_scalar_sub(sm, oh, mx)
            e1 = p0.tile([128, c.dh2], F32, tag="e1")
            nc.scalar.activation(e1, sm, AF.Exp)
            se = p0.tile([128, 1], F32, tag="se")
            nc.vector.tensor_reduce(se, e1, mybir.AxisListType.X, ALU.add)
            ln = p0.tile([128, 1], F32, tag="ln")
            nc.scalar.activation(ln, se, AF.Ln)
            nc.vector.tensor_scalar_sub(sm, sm, ln)
            nc.sync.dma_start(olv[:, b, :], sm)

        edge_phase(t2_full, t2_loc, c.dh2, c.dt2, post2)


# ----------------------------------------------------------------------------
# Host entry point
# ----------------------------------------------------------------------------

_CACHE = {}


def _get_compiled(cfg: Cfg, plan: Plan):
    key = (cfg, plan)
    if key not in _CACHE:
        nc = bacc.Bacc(
            "TRN2", target_bir_lowering=False, debug=False,
            num_devices=cfg.cores, num_swdge_queues=4,
        )
        with tile.TileContext(nc) as tc:
            build(nc, tc, cfg, plan)
        nc.compile()
        _CACHE[key] = nc
    return _CACHE[key]


def make_in_maps(cfg: Cfg, x, W1, b1, W2, b2, deg_pt, idx16, ids_f32, cnts_wm):
    import ml_dtypes

    c = cfg
    x = np.asarray(x, np.float32)
    w2p = np.asarray(W2, np.float32)[:, : c.dh2].astype(ml_dtypes.bfloat16)
    b1rep = np.tile(np.asarray(b1, np.float32)[None, :], (128, 1))
    b2rep = np.tile(np.asarray(b2, np.float32)[None, : c.dh2], (128, 1))
    w1c = np.ascontiguousarray(
        np.asarray(W1, np.float32).astype(ml_dtypes.bfloat16)
    )

    in_maps = []
    for ci in range(c.cores):
        xs = np.zeros((c.din, c.nloc), ml_dtypes.bfloat16)
        xs[:, : c.nsh] = (
            x[ci * c.nsh : (ci + 1) * c.nsh].astype(ml_dtypes.bfloat16).T
        )
        in_maps.append({
            "x_sh": np.ascontiguousarray(xs),
            "w1": w1c,
            "w2": np.ascontiguousarray(w2p),
            "b1r": b1rep,
            "b2r": np.ascontiguousarray(b2rep),
            "degp": np.ascontiguousarray(deg_pt[ci]),
            "idx16": np.ascontiguousarray(idx16[ci]),
            "idsf": np.ascontiguousarray(ids_f32[ci].astype(ml_dtypes.bfloat16)),
            "cnts32": np.ascontiguousarray(
                np.tile(cnts_wm[ci][None, :], (128, 1))
            ),
        })
    return in_maps


def _ensure_ntff_hook():
    """Install the axon NTFF profile hook if the image's antenv lacks it."""
    import types

    try:
        from antenv.axon_hooks import get_axon_ntff_profile_hook  # noqa: F401
        return
    except ImportError:
        pass
    import antenv

    m = types.ModuleType("antenv.axon_hooks")
    m._hook = None
    m.set_axon_ntff_profile_hook = lambda h: setattr(m, "_hook", h)
    m.get_axon_ntff_profile_hook = lambda: m._hook
    sys.modules["antenv.axon_hooks"] = m
    antenv.axon_hooks = m
    try:
        from trn_agent_boot.trn_boot import _ntff_profile_via_ctypes

        h = _ntff_profile_via_ctypes("/opt/axon/libaxon_pjrt.so")
        if h is not None:
            m._hook = h
    except Exception as e:
        print(f"ntff hook install failed: {e}")

    from concourse import bass_utils as bu

    bu.upload_artifacts = lambda tmpdir: tmpdir


def run(cfg: Cfg, inputs: dict, trace: bool = False):
    if trace:
        _ensure_ntff_hook()
    deg_pt, idx16, ids_f32, cnts_wm, plan = preprocess(cfg, inputs["edge_index"])
    nc = _get_compiled(cfg, plan)
    in_maps = make_in_maps(
        cfg, inputs["x"], inputs["W1"], inputs["b1"], inputs["W2"], inputs["b2"],
        deg_pt, idx16, ids_f32, cnts_wm,
    )
    res = run_bass_kernel_spmd(
        nc, in_maps, core_ids=list(range(cfg.cores)), trace=trace
    )
    c = cfg
    h = np.concatenate(
        [res.results[ci]["out_h"][: c.nsh, : c.dout] for ci in range(c.cores)], axis=0
    )
    ls = np.concatenate(
        [res.results[ci]["out_ls"][: c.nsh, : c.dout] for ci in range(c.cores)], axis=0
    )
    return (h, ls), res


def kernel(**inputs):
    (h, ls), _ = run(Cfg(), inputs)
    return h, ls
